# revision 1
# baseline (speedup 1.0000x reference)
"""Trainium2 Bass kernel for causal multi-head attention with RoPE.

Problem: B=2, S=2048, DIM=1024, 16 heads, head_dim=64.
  q = x @ Wq.T ; k = x @ Wk.T ; v = x @ Wv.T        (torch Linear convention)
  q, k = rope(q), rope(k)                            (Llama interleaved pairs)
  y = softmax(causal(q k^T / 8)) v @ Wo.T

Sharding (8 cores): data-parallel over batch (2) x tensor-parallel over
head groups (4 heads per core).  Wq/Wk/Wv row-sharded, Wo column-sharded;
the host sums the 4 partial outputs per batch.

Per-core dataflow (everything "transposed": features on partitions,
tokens on the free axis), all matmul operands fp16, accumulation fp32:

  phase 1 (PE-dense; warms the PE clock gate):
    XT [1024, S] <- host-transposed fp16
    QT/KT = W @ XT -> PSUM -> fp16 + RoPE on DVE (host-baked cos/sin
      tables; head dims de-interleaved by a host permutation of W rows so
      the RoPE pair swap is a +-32 partition copy)
    V = XT^T-blocks @ WvT -> [S, 256] fp16 with a ones column per head
      ([V | 1]) so P @ [V|1] also produces the softmax denominator.

  phase 2 (ScalarE-bound; ST for block j+1 pipelined against PV for j):
    ST_j = scores^T [128 k, q] in PSUM, two heads row-packed on the PE
    PT_j = exp(0.125 * ST_j) on ScalarE (fp16; no max subtraction --
      scores are O(1) here), causal 0/1 mask multiply on diagonal blocks
      only, and the diagonal 128-col PV chunk is emitted separately so
      the mask multiply is off the critical path.
    OB_h [65, q] += [V_j | 1]^T-stationary @ PT_j  (PSUM, accum over j)
    normalize: l = row 64; 1/l via reciprocal_approx_fast (SBUF),
    GPSIMD partition-broadcast, one DVE multiply -> ZT fp16 (Z^T layout,
    no transposes needed).

  phase 3: YT = WoT^T-stationary @ ZT -> fp16 partial out, DMA'd out.
"""

import os
import sys

sys.path.insert(0, "/opt/trn_rl_repo")

import numpy as np

import concourse.bass as bass
import concourse.mybir as mybir
import concourse.tile as tile
from concourse import bacc
from concourse.bass_utils import run_bass_kernel_spmd

F16 = mybir.dt.float16
F32 = mybir.dt.float32

DIM = 1024
NUM_HEADS = 16
HEAD_DIM = 64
B = 2
GROUPS = 4                   # head groups (tensor parallel)
HPG = NUM_HEADS // GROUPS    # heads per group = 4
FG = HPG * HEAD_DIM          # features per group = 256
THETA = 10000.0


def build_program(S=2048):
    from contextlib import ExitStack

    nc = bacc.Bacc(None, target_bir_lowering=False)
    NT = S // 128                 # token blocks
    QTILE = min(1024, S)
    NQT = S // QTILE

    xt_d = nc.declare_dram_parameter("xt", [DIM, S], F16, isOutput=False)
    wq_d = nc.declare_dram_parameter("wqt", [DIM, FG], F16, isOutput=False)
    wk_d = nc.declare_dram_parameter("wkt", [DIM, FG], F16, isOutput=False)
    wv_d = nc.declare_dram_parameter("wvt", [DIM, FG], F16, isOutput=False)
    wo_d = nc.declare_dram_parameter("wot", [FG, DIM], F16, isOutput=False)
    cos_d = nc.declare_dram_parameter("cos", [128, S], F16, isOutput=False)
    sin_d = nc.declare_dram_parameter("sins", [128, S], F16, isOutput=False)
    mask_d = nc.declare_dram_parameter("mask", [128, 128], F16, isOutput=False)
    # fp16 partial output; host upcasts to fp32 before summing the 4 partials
    yt_d = nc.declare_dram_parameter("yt", [DIM, S], F16, isOutput=True)

    Exp = mybir.ActivationFunctionType.Exp

    with tile.TileContext(nc) as tc:
        with ExitStack() as ctx:
            consts = ctx.enter_context(tc.tile_pool(name="consts", bufs=1))

            qt_sb = consts.tile([128, 2, S], F16)
            kt_sb = consts.tile([128, 2, S], F16)
            vaug = consts.tile([128, NT, HPG * 65], F16)
            zt_sb = consts.tile([128, 2, S], F16)

            # ---------- schedule ----------
            # phase 1: projections, PE-dense (k-outer loops reuse the
            #   stationary weight block across 4 moving tiles), warms HAM
            # phase 2: attention, per-(qtile, head) units, 3-deep ST
            #   lookahead so ScalarE exp streams without bubbles
            # phase 3: output projection
            from contextlib import ExitStack as _ES

            with _ES() as pctx:
                wpool = pctx.enter_context(tc.tile_pool(name="wpool", bufs=1))
                wq_sb = wpool.tile([128, 8, FG], F16)
                nc.sync.dma_start(wq_sb[:], wq_d[:].rearrange("(c p) f -> p c f", p=128))
                wk_sb = wpool.tile([128, 8, FG], F16)
                nc.sync.dma_start(wk_sb[:], wk_d[:].rearrange("(c p) f -> p c f", p=128))
                wv_sb = wpool.tile([128, 8, FG], F16)
                nc.sync.dma_start(wv_sb[:], wv_d[:].rearrange("(c p) f -> p c f", p=128))
                xt_sb = wpool.tile([128, 8, S], F16)
                xt_r = xt_d[:].rearrange("(c p) t -> p c t", p=128)
                for k in range(8):
                    nc.sync.dma_start(xt_sb[:, k, 0:S // 2], xt_r[:, k, 0:S // 2])
                    nc.sync.dma_start(xt_sb[:, k, S // 2:S], xt_r[:, k, S // 2:S])
                cos_t = consts.tile([128, S], F16)
                nc.sync.dma_start(cos_t[:], cos_d[:])
                sin_t = consts.tile([128, S], F16)
                nc.sync.dma_start(sin_t[:], sin_d[:])
                mask_t = consts.tile([128, 128], F16)
                nc.sync.dma_start(mask_t[:], mask_d[:])
                wo_sb = consts.tile([128, 2, DIM], F16)
                nc.sync.dma_start(wo_sb[:], wo_d[:].rearrange("(c p) d -> p c d", p=128))
                nc.vector.memset(vaug[:], 1.0)

                pp = pctx.enter_context(tc.tile_pool(name="psproj", bufs=2, space="PSUM"))
                rp = pctx.enter_context(tc.tile_pool(name="rope", bufs=6))

                # Q/K projections + RoPE; k outer so each weight block is
                # loaded once and streamed against 4 moving tiles
                for wsb, dest, c in (
                    (wq_sb, qt_sb, 0),
                    (wk_sb, kt_sb, 0),
                    (wq_sb, qt_sb, 1),
                    (wk_sb, kt_sb, 1),
                ):
                    if True:
                        psq = pp.tile([128, S], F32, tag="proj", name=f"psq_{c}")
                        for k in range(8):
                            for t4 in range(S // 512):
                                nc.tensor.matmul(
                                    psq[:, t4 * 512:(t4 + 1) * 512],
                                    lhsT=wsb[:, k, c * 128:(c + 1) * 128],
                                    rhs=xt_sb[:, k, t4 * 512:(t4 + 1) * 512],
                                    start=(k == 0),
                                    stop=(k == 7),
                                )
                        qc = rp.tile([128, S], F16, tag="rope")
                        nc.scalar.copy(qc[:], psq[:])  # cast fp32->fp16 on ACT
                        t1 = rp.tile([128, S], F16, tag="rope")
                        nc.vector.tensor_mul(t1[:], qc[:], cos_t[:])
                        rot = rp.tile([128, S], F16, tag="rope")
                        for qq in range(4):
                            srcp = (qq ^ 1) * 32
                            nc.vector.tensor_copy(
                                rot[qq * 32:(qq + 1) * 32, :], qc[srcp:srcp + 32, :]
                            )
                        t2 = rp.tile([128, S], F16, tag="rope")
                        nc.vector.tensor_mul(t2[:], rot[:], sin_t[:])
                        nc.vector.tensor_add(dest[:, c, :], t1[:], t2[:])

                # V projection: V[tok, f] so the PV contraction has tokens
                # on partitions
                for tb in range(NT):
                    psv = pp.tile([128, FG], F32, tag="proj", name=f"psv_{tb}")
                    for k in range(8):
                        nc.tensor.matmul(
                            psv[:],
                            lhsT=xt_sb[:, k, tb * 128:(tb + 1) * 128],
                            rhs=wv_sb[:, k, :],
                            start=(k == 0),
                            stop=(k == 7),
                        )
                    nc.vector.tensor_copy(
                        vaug[:, tb, :].rearrange("p (h c) -> p h c", c=65)[:, :, 0:64],
                        psv[:].rearrange("p (h d) -> p h d", d=64),
                    )

            # ---------------- phase 2: attention ----------------
            # (qtile, head-pair) units; scores for k-block j+1 emitted
            # between the PV matmuls for block j; tails pairwise interleaved
            with _ES() as actx:
                stp = actx.enter_context(tc.tile_pool(name="stps", bufs=2, space="PSUM"))
                op_ = actx.enter_context(tc.tile_pool(name="ops", bufs=2, space="PSUM"))
                ptp = actx.enter_context(tc.tile_pool(name="ptp", bufs=16))
                sm = actx.enter_context(tc.tile_pool(name="smp", bufs=4))
                bp = actx.enter_context(tc.tile_pool(name="bcp", bufs=4))

                for qt_i in range(NQT):
                    qlo = qt_i * QTILE
                    jmax = (qlo + QTILE) // 128
                    for pair in range(2):
                        pts = {}
                        obs = {}

                        def emit_st(j, pair=pair, qlo=qlo, pts=pts):
                            qs = max(qlo, j * 128)
                            w = qlo + QTILE - qs
                            for hh in range(2):
                                base = 64 * hh
                                st = stp.tile([128, QTILE], F32, tag="st")
                                for nn in range(0, w, 512):
                                    ww = min(512, w - nn)
                                    nc.tensor.matmul(
                                        st[:, nn:nn + ww],
                                        lhsT=kt_sb[base:base + 64, pair, j * 128:(j + 1) * 128],
                                        rhs=qt_sb[base:base + 64, pair, qs + nn:qs + nn + ww],
                                        start=True,
                                        stop=True,
                                    )
                                pt = ptp.tile([128, QTILE], F16, tag="pt")
                                nc.scalar.activation(pt[:, 0:w], st[:, 0:w], Exp, scale=0.125)
                                if j * 128 >= qlo:
                                    nc.vector.tensor_mul(pt[:, 0:128], pt[:, 0:128], mask_t[:])
                                pts[(hh, j)] = (pt, qs)

                        def emit_pv(j, pair=pair, qlo=qlo, pts=pts, obs=obs):
                            for hh in range(2):
                                hg = pair * 2 + hh
                                pt, qs = pts.pop((hh, j))
                                c0 = qs - qlo
                                while c0 < QTILE:
                                    c1 = min((c0 // 512 + 1) * 512, QTILE)
                                    last_j = (qlo + c1 - 1) // 128
                                    nc.tensor.matmul(
                                        obs[hh][:, c0:c1],
                                        lhsT=vaug[:, j, hg * 65:(hg + 1) * 65],
                                        rhs=pt[:, c0 - (qs - qlo):c1 - (qs - qlo)],
                                        start=(j == 0),
                                        stop=(j == last_j),
                                        skip_group_check=True,
                                    )
                                    c0 = c1

                        for hh in range(2):
                            obs[hh] = op_.tile(
                                [65, QTILE], F32, tag="o", name=f"ob_{qt_i}_{pair}_{hh}"
                            )
                        emit_st(0)
                        for j in range(jmax):
                            if j + 1 < jmax:
                                emit_st(j + 1)
                            emit_pv(j)
                        lrows = {}
                        rcps = {}
                        bcs = {}
                        for hh in range(2):
                            lrows[hh] = sm.tile(
                                [1, QTILE], F32, tag="lrow", name=f"lr_{qt_i}_{pair}_{hh}"
                            )
                            nc.vector.tensor_copy(lrows[hh][:], obs[hh][64:65, :])
                        for hh in range(2):
                            rcps[hh] = sm.tile(
                                [1, QTILE], F32, tag="rcp", name=f"rc_{qt_i}_{pair}_{hh}"
                            )
                            nc.vector.reciprocal_approx_fast(rcps[hh][:], lrows[hh][:])
                        for hh in range(2):
                            bcs[hh] = bp.tile(
                                [64, QTILE], F32, tag="bc", name=f"bc_{qt_i}_{pair}_{hh}"
                            )
                            nc.gpsimd.partition_broadcast(bcs[hh][:], rcps[hh][:])
                        for hh in range(2):
                            nc.vector.tensor_mul(
                                zt_sb[hh * 64:(hh + 1) * 64, pair, qlo:qlo + QTILE],
                                obs[hh][0:64, :],
                                bcs[hh][:],
                            )

            # ---------------- phase 3: output projection ----------------
            with _ES() as octx:
                pyp = octx.enter_context(tc.tile_pool(name="psy", bufs=2, space="PSUM"))
                yp = octx.enter_context(tc.tile_pool(name="ysb", bufs=3))
                yt_r = yt_d[:].rearrange("(c p) t -> p c t", p=128)
                for dchunk in range(8):
                    for th in range(S // 1024):
                        psy = pyp.tile([128, 1024], F32, tag="psy")
                        for nn in range(2):
                            for c2 in range(2):
                                nc.tensor.matmul(
                                    psy[:, nn * 512:(nn + 1) * 512],
                                    lhsT=wo_sb[:, c2, dchunk * 128:(dchunk + 1) * 128],
                                    rhs=zt_sb[:, c2, th * 1024 + nn * 512:th * 1024 + (nn + 1) * 512],
                                    start=(c2 == 0),
                                    stop=(c2 == 1),
                                )
                        yt_sb = yp.tile([128, 1024], F16, tag="y")
                        if (dchunk + th) % 2 == 0:
                            nc.vector.tensor_copy(yt_sb[:], psy[:])
                        else:
                            nc.scalar.copy(yt_sb[:], psy[:])
                        nc.sync.dma_start(
                            yt_r[:, dchunk, th * 1024:(th + 1) * 1024], yt_sb[:]
                        )

    nc.compile()
    return nc


def _host_inputs(x, Wq, Wk, Wv, Wo, S):
    """Per-core input maps (host-side sharding + layout prep)."""
    # de-interleave RoPE pairs within each head: (2i, 2i+1) -> (i, i+32)
    perm = np.concatenate([np.arange(0, HEAD_DIM, 2), np.arange(1, HEAD_DIM, 2)])
    rp = (np.arange(HPG)[:, None] * HEAD_DIM + perm[None, :]).reshape(-1)

    half = HEAD_DIM // 2
    inv_freq = THETA ** (-np.arange(half, dtype=np.float64) * 2.0 / HEAD_DIM)
    ang = np.arange(S, dtype=np.float64)[None, :] * inv_freq[:, None]  # [32, S]
    cos32 = np.cos(ang)
    sin32 = np.sin(ang)
    cos128 = np.tile(cos32, (4, 1)).astype(np.float16)
    sins128 = np.concatenate([-sin32, sin32, -sin32, sin32], axis=0).astype(np.float16)
    mask = (np.arange(128)[None, :] >= np.arange(128)[:, None]).astype(np.float16)

    in_maps = []
    for core in range(B * GROUPS):
        b, g = divmod(core, GROUPS)
        sl = slice(g * FG, (g + 1) * FG)
        in_maps.append(
            dict(
                xt=np.ascontiguousarray(x[b].T).astype(np.float16),
                wqt=np.ascontiguousarray(Wq[sl][rp].T).astype(np.float16),
                wkt=np.ascontiguousarray(Wk[sl][rp].T).astype(np.float16),
                wvt=np.ascontiguousarray(Wv[sl].T).astype(np.float16),
                wot=np.ascontiguousarray(Wo[:, sl].T).astype(np.float16),
                cos=cos128,
                sins=sins128,
                mask=mask,
            )
        )
    return in_maps


def _install_ntff_hook():
    """Provide antenv.axon_hooks if the image lacks it (NTFF profiling
    under axon; mirrors trn_agent_boot._ntff_profile_via_ctypes)."""
    try:
        from antenv.axon_hooks import get_axon_ntff_profile_hook  # noqa: F401
        return
    except ImportError:
        pass
    import contextlib
    import ctypes
    import types

    so_path = "/opt/axon/libaxon_pjrt.so"
    if not os.path.exists(so_path):
        return
    lib = ctypes.CDLL(so_path)
    if not hasattr(lib, "axon_start_nrt_profile"):
        return
    lib.axon_start_nrt_profile.argtypes = [
        ctypes.POINTER(ctypes.c_int64),
        ctypes.c_size_t,
    ]
    lib.axon_start_nrt_profile.restype = ctypes.c_int64
    lib.axon_stop_nrt_profile.argtypes = [ctypes.c_char_p]
    lib.axon_stop_nrt_profile.restype = ctypes.c_int64

    @contextlib.contextmanager
    def _hook(output_dir, device_ids):
        import jax

        jax.devices()
        if device_ids:
            ids = (ctypes.c_int64 * len(device_ids))(*device_ids)
            rc = lib.axon_start_nrt_profile(ids, len(device_ids))
        else:
            rc = lib.axon_start_nrt_profile(None, 0)
        if rc != 0:
            raise RuntimeError(f"axon_start_nrt_profile rc={rc}")
        try:
            yield
        finally:
            n = lib.axon_stop_nrt_profile(str(output_dir).encode())
            print(f"profile: {n} file(s) written to {output_dir}")

    mod = types.ModuleType("antenv.axon_hooks")
    _state = {"hook": _hook}
    mod.get_axon_ntff_profile_hook = lambda: _state["hook"]
    mod.set_axon_ntff_profile_hook = lambda h: _state.__setitem__("hook", h)
    import antenv

    antenv.axon_hooks = mod
    sys.modules["antenv.axon_hooks"] = mod


_NC_CACHE = {}


def _get_nc(S):
    if S not in _NC_CACHE:
        _NC_CACHE[S] = build_program(S)
    return _NC_CACHE[S]


def kernel(x, Wq, Wk, Wv, Wo, _trace=False, _tmpdir=None):
    x = np.asarray(x, dtype=np.float32)
    Wq = np.asarray(Wq, dtype=np.float32)
    Wk = np.asarray(Wk, dtype=np.float32)
    Wv = np.asarray(Wv, dtype=np.float32)
    Wo = np.asarray(Wo, dtype=np.float32)
    S = x.shape[1]

    if _trace:
        _install_ntff_hook()
    nc = _get_nc(S)
    in_maps = _host_inputs(x, Wq, Wk, Wv, Wo, S)
    res = run_bass_kernel_spmd(
        nc, in_maps, core_ids=list(range(8)), trace=_trace, tmpdir=_tmpdir
    )
    yts = [res.results[c]["yt"].astype(np.float32) for c in range(8)]
    y = np.stack(
        [sum(yts[b * GROUPS + g] for g in range(GROUPS)).T for b in range(B)]
    ).astype(np.float32)
    if _trace:
        kernel.last_results = res
    return y

